# revision 14
# baseline (speedup 1.0000x reference)
"""Trainium2 Bass kernel for nn_AttnDecoderRNN (B=64,S=128,H=1024,V=32000,T=10).

Strategy (8 NeuronCores, SPMD, one NEFF):
- Recurrence (attention + GRU) batch-sharded: 8 batches/core, zero cross-core
  traffic during the 10 steps.
- Output projection vocab-sharded: 4000 vocab rows/core over all 640 (b,t)
  rows, with one AllGather (hidden states) + one AllReduce (softmax sums).
- bf16 matmuls with fp32 PSUM accumulation; log_softmax without max-shift
  (logits are O(1); exp is fp32-safe).
"""
import os
import sys

for p in ("/opt/trn_rl_repo", "/root/.axon_site/_ro/trn_rl_repo"):
    if os.path.isdir(p) and p not in sys.path:
        sys.path.insert(0, p)

import numpy as np
import ml_dtypes

import concourse.bass as bass
import concourse.mybir as mybir
import concourse.bacc as bacc
import concourse.tile as tile
import concourse.bass_isa as bass_isa
from concourse.bass_utils import run_bass_kernel_spmd

BF = mybir.dt.bfloat16
F32 = mybir.dt.float32
NPBF = ml_dtypes.bfloat16

B, S, H, V, T = 64, 128, 1024, 32000, 10
NC = 8
BL = B // NC          # 8 local batches per core
VS = V // NC          # 4000 vocab rows per core
KC = H // 128         # 8 k-chunks of hidden dim
MT = (B * T) // 128   # 5 row-tiles of the 640 global (b,t) rows
NCH = 8               # vocab-shard n-chunks
NW = VS // NCH        # 500 columns per chunk

ACT = mybir.ActivationFunctionType
ALU = mybir.AluOpType

_CACHE = {}
K_CC = os.environ.get("K_CC", "1") == "1"



def _bf(x):
    return np.ascontiguousarray(x).astype(NPBF)


def _build_program():
    nc = bacc.Bacc("TRN2", target_bir_lowering=False, debug=False, num_devices=NC)

    def din(name, shape, dt=BF):
        return nc.dram_tensor(name, list(shape), dt, kind="ExternalInput").ap()

    def dout(name, shape, dt=F32):
        return nc.dram_tensor(name, list(shape), dt, kind="ExternalOutput").ap()

    encT_d = din("encT", [BL, H, S])            # Uk lhsT: [b][k][s]
    encN_d = din("encN", [BL, S, H])            # ctx rhs: [b][s][h]
    UaT_d = din("UaT", [H, H])
    Uab_d = din("Uab", [1, H])                  # Ua_b + Wa_b folded (bf16)
    WaT_d = din("WaT", [H, H])
    Va_d = din("Va", [1, H])
    W1T_d = din("W1T", [H, 3 * H])
    bemb_d = din("bemb", [1, 3 * H])            # b_ih+b_hh (rz), b_ih (n) (bf16)
    WrzT_d = din("WrzT", [2 * H, 2 * H])
    WncT_d = din("WncT", [H, H])
    WnhT_d = din("WnhT", [H, H])
    bhn_d = din("bhn", [1, H], F32)             # b_hh (n gate)
    outWT_d = din("outWT", [H, VS])
    outb_d = din("outb", [1, VS], F32)
    embT_d = din("embT", [128, KC, BL * T])     # pre-gathered embeddings
    h0T_d = din("h0T", [128, KC, BL])
    h0_d = din("h0", [BL, H], F32)
    ident_d = din("ident", [128, 128], F32)
    identb_d = din("identb", [128, 128])
    bsel_d = din("bsel", [128, BL, 128])

    logp_d = dout("logp_s", [B * T, VS])        # rows: src*80 + t*8 + b
    hout_d = dout("h_out", [BL, H])
    attn_d = dout("attn_out", [BL, T, S])

    with tile.TileContext(nc) as tc:
        with tc.tile_pool(name="dram", bufs=1, space="DRAM") as dram, \
             tc.tile_pool(name="dataX", bufs=1) as dataX, \
             tc.tile_pool(name="psum", bufs=2, space="PSUM") as psum, \
             tc.tile_pool(name="ptr", bufs=2, space="PSUM") as ptr:

            hg_in = dram.tile([128, KC, BL * T], BF)
            hg_out = nc.dram_tensor("hg_out", [NC * 128, KC, BL * T], BF, addr_space="Shared").ap()
            st_in = dram.tile([128, MT], F32)
            st_out = nc.dram_tensor("st_out", [128, MT], F32, addr_space="Shared").ap()
            uk_d = dram.tile([BL, 128, H], BF)

            HTall_s = dataX.tile([128, KC, BL * T], BF)
            attnb_s = dataX.tile([128, T, BL], F32)
            hm_s = dataX.tile([BL, H], F32)
            ident_s = dataX.tile([128, 128], F32)
            stage_s = dataX.tile([128, 128], F32)   # padded transpose input

            nc.sync.dma_start(ident_s[:], ident_d[:])
            nc.sync.dma_start(hm_s[:], h0_d[:])
            nc.vector.memset(stage_s[:], 0.0)

            def load_T(sb, dr, kc):  # [kc*128+p, n] dram -> [p, kc, n] sbuf
                for c in range(kc):
                    nc.sync.dma_start(sb[:, c, :], dr[c * 128:(c + 1) * 128, :])

            with tc.tile_pool(name="ph2w", bufs=1) as ph2w, \
                 tc.tile_pool(name="data2", bufs=1) as data2:
                WaT_s = ph2w.tile([128, KC, H], BF)
                WrzT_s = ph2w.tile([128, 2 * KC, 2 * H], BF)
                WncT_s = ph2w.tile([128, KC, H], BF)
                WnhT_s = ph2w.tile([128, KC, H], BF)
                Va_s = ph2w.tile([128, H], BF)
                bhn_s = ph2w.tile([128, H], F32)
                embT_s = ph2w.tile([128, KC, BL * T], BF)

                load_T(WaT_s, WaT_d, KC)
                load_T(WrzT_s, WrzT_d, 2 * KC)
                load_T(WncT_s, WncT_d, KC)
                load_T(WnhT_s, WnhT_d, KC)
                nc.sync.dma_start(Va_s[:], Va_d[0:1, :].partition_broadcast(128))
                nc.sync.dma_start(bhn_s[:], bhn_d[0:1, :].partition_broadcast(128))
                nc.sync.dma_start(embT_s[:], embT_d[:])

                encN_s = data2.tile([128, BL, H], BF)
                identb_s = data2.tile([128, 128], BF)
                bsel_s = data2.tile([128, BL, 128], BF)
                gie_s = data2.tile([128, 3 * H], BF)
                xhT_s = data2.tile([128, 2 * KC, 128], BF)
                ctxp_s = data2.tile([128, H], F32)
                wsel_s = data2.tile([128, BL, 128], BF)

                for b in range(BL):
                    nc.sync.dma_start(encN_s[:, b, :], encN_d[b])
                nc.sync.dma_start(identb_s[:], identb_d[:])
                nc.sync.dma_start(bsel_s[:], bsel_d[:])
                nc.sync.dma_start(xhT_s[:, KC:2 * KC, 0:BL], h0T_d[:])
                nc.vector.memset(xhT_s[:, 0:KC, :], 0.0)
                nc.vector.memset(xhT_s[:, KC:2 * KC, BL:128], 0.0)
                nc.vector.memset(ctxp_s[:], 0.0)
                nc.vector.memset(wsel_s[:], 0.0)

                # ---- phase 1: Uk (to DRAM) + gi_emb ----
                with tc.tile_pool(name="ph1", bufs=2) as ph1, \
                     tc.tile_pool(name="ph1w", bufs=1) as ph1w, \
                     tc.tile_pool(name="ph1s", bufs=4) as ph1s:
                    UaT_s = ph1w.tile([128, KC, H], BF)
                    Uab_s = ph1w.tile([128, H], BF)
                    bemb_s = ph1w.tile([128, 3 * H], BF)
                    load_T(UaT_s, UaT_d, KC)
                    nc.sync.dma_start(Uab_s[:], Uab_d[0:1, :].partition_broadcast(128))
                    nc.sync.dma_start(bemb_s[:], bemb_d[0:1, :].partition_broadcast(128))
                    for b in range(BL):
                        eT = ph1.tile([128, KC, S], BF, tag="encT")
                        for c in range(KC):
                            nc.sync.dma_start(
                                eT[:, c, :], encT_d[b, c * 128:(c + 1) * 128, :])
                        ps = psum.tile([128, H], F32, tag="mm")
                        for c in range(KC):
                            for n2 in range(2):
                                nc.tensor.matmul(
                                    ps[:, n2 * 512:(n2 + 1) * 512],
                                    lhsT=eT[:, c, :],
                                    rhs=UaT_s[:, c, n2 * 512:(n2 + 1) * 512],
                                    start=(c == 0), stop=(c == KC - 1))
                        uksb = ph1.tile([128, H], BF, tag="uksb")
                        nc.vector.tensor_tensor(
                            uksb[:], ps[:], Uab_s[:], ALU.add)
                        nc.sync.dma_start(uk_d[b], uksb[:])
                    # gi_emb[80, 3072] = embT.T @ W1T + bemb
                    for n3 in range(3):
                        ps = psum.tile([128, H], F32, tag="mm")
                        for half in range(2):
                            col0 = n3 * 1024 + half * 512
                            for c in range(KC):
                                w1 = ph1s.tile([128, 512], BF, tag="w1")
                                nc.sync.dma_start(
                                    w1[:], W1T_d[c * 128:(c + 1) * 128, col0:col0 + 512])
                                nc.tensor.matmul(
                                    ps[0:BL * T, half * 512:(half + 1) * 512],
                                    lhsT=embT_s[:, c, :],
                                    rhs=w1[:],
                                    start=(c == 0), stop=(c == KC - 1))
                        nc.vector.tensor_tensor(
                            gie_s[0:BL * T, n3 * 1024:(n3 + 1) * 1024],
                            ps[0:BL * T, :],
                            bemb_s[0:BL * T, n3 * 1024:(n3 + 1) * 1024],
                            ALU.add)

                # ---- phase 2: 10 recurrence steps ----
                with tc.tile_pool(name="step", bufs=2) as step, \
                     tc.tile_pool(name="uks", bufs=3) as uks, \
                     tc.tile_pool(name="sm", bufs=1) as sm:
                    for t in range(T):
                        # q = h @ Wa^T
                        psq = psum.tile([128, H], F32, tag="mm")
                        for c in range(KC):
                            for n2 in range(2):
                                nc.tensor.matmul(
                                    psq[:, n2 * 512:(n2 + 1) * 512],
                                    lhsT=xhT_s[:, KC + c, :],
                                    rhs=WaT_s[:, c, n2 * 512:(n2 + 1) * 512],
                                    start=(c == 0), stop=(c == KC - 1))
                        q_sb = sm.tile([128, H], BF, tag="q")
                        nc.vector.tensor_copy(q_sb[:], psq[:])

                        scores = sm.tile([128, BL], F32, tag="sc")
                        for b in range(BL):
                            ukt = uks.tile([128, H], BF, tag="uk")
                            nc.sync.dma_start(ukt[:], uk_d[b])
                            pst = psum.tile([128, H], F32, tag="mm")
                            for n2 in range(2):
                                sl = slice(n2 * 512, (n2 + 1) * 512)
                                nc.tensor.matmul(
                                    pst[:, sl], lhsT=identb_s[:], rhs=ukt[:, sl],
                                    start=True, stop=False)
                                nc.tensor.matmul(
                                    pst[:, sl], lhsT=bsel_s[:, b, :], rhs=q_sb[:, sl],
                                    start=False, stop=True)
                            tt = step.tile([128, H], BF, tag="tt")
                            nc.scalar.activation(tt[:], pst[:], ACT.Tanh)
                            th = step.tile([128, H], BF, tag="th")
                            nc.vector.scalar_tensor_tensor(
                                out=th[:], in0=tt[:], scalar=1.0, in1=Va_s[:],
                                op0=ALU.mult, op1=ALU.mult,
                                accum_out=scores[:, b:b + 1])
                        # softmax over s (partitions) on gpsimd
                        smax = sm.tile([128, BL], F32, tag="smax")
                        nc.gpsimd.partition_all_reduce(
                            smax[:], scores[:], 128, bass_isa.ReduceOp.max)
                        esb = sm.tile([128, BL], F32, tag="esb")
                        nc.vector.tensor_sub(esb[:], scores[:], smax[:])
                        nc.scalar.activation(esb[:], esb[:], ACT.Exp)
                        ssum = sm.tile([128, BL], F32, tag="ssum")
                        nc.gpsimd.partition_all_reduce(
                            ssum[:], esb[:], 128, bass_isa.ReduceOp.add)
                        rcp = sm.tile([128, BL], F32, tag="rcp")
                        nc.vector.reciprocal(rcp[:], ssum[:])
                        nc.vector.tensor_mul(attnb_s[:, t, :], esb[:], rcp[:])
                        for b in range(BL):
                            nc.vector.tensor_copy(
                                wsel_s[:, b, b:b + 1], attnb_s[:, t, b:b + 1])

                        # ctx: 8 accumulating matmuls; selector column b puts
                        # w_b @ encN[b] in psum row b, zeros elsewhere.
                        psc = psum.tile([128, H], F32, tag="mm")
                        for b in range(BL):
                            for n2 in range(2):
                                nc.tensor.matmul(
                                    psc[:, n2 * 512:(n2 + 1) * 512],
                                    lhsT=wsel_s[:, b, :],
                                    rhs=encN_s[:, b, n2 * 512:(n2 + 1) * 512],
                                    start=(b == 0), stop=(b == BL - 1))
                        nc.vector.tensor_copy(ctxp_s[0:BL, :], psc[0:BL, :])
                        for c in range(KC):
                            pt = ptr.tile([128, 128], F32, tag="tr")
                            nc.tensor.transpose(
                                pt[:], ctxp_s[:, c * 128:(c + 1) * 128], ident_s[:])
                            nc.vector.tensor_copy(xhT_s[:, c, 0:BL], pt[:, 0:BL])

                        # GRU gates
                        gie_t = sm.tile([BL, 3 * H], BF, tag="gie_t")
                        nc.sync.dma_start(gie_t[:], gie_s[t * BL:(t + 1) * BL, :])
                        grz = sm.tile([BL, 2 * H], BF, tag="grz")
                        for h2 in range(2):
                            ps = psum.tile([128, H], F32, tag="mm")
                            for c in range(2 * KC):
                                for n2 in range(2):
                                    col = h2 * H + n2 * 512
                                    nc.tensor.matmul(
                                        ps[:, n2 * 512:(n2 + 1) * 512],
                                        lhsT=xhT_s[:, c, :],
                                        rhs=WrzT_s[:, c, col:col + 512],
                                        start=(c == 0), stop=(c == 2 * KC - 1))
                            nc.vector.tensor_tensor(
                                grz[:, h2 * H:(h2 + 1) * H], ps[0:BL, :],
                                gie_t[:, h2 * H:(h2 + 1) * H],
                                ALU.add)
                        psn = psum.tile([128, H], F32, tag="mm")
                        for c in range(KC):
                            for n2 in range(2):
                                nc.tensor.matmul(
                                    psn[:, n2 * 512:(n2 + 1) * 512],
                                    lhsT=xhT_s[:, c, :],
                                    rhs=WncT_s[:, c, n2 * 512:(n2 + 1) * 512],
                                    start=(c == 0), stop=(c == KC - 1))
                        inn = sm.tile([BL, H], BF, tag="inn")
                        nc.vector.tensor_tensor(
                            inn[:], psn[0:BL, :],
                            gie_t[:, 2 * H:3 * H], ALU.add)
                        psh = psum.tile([128, H], F32, tag="mm")
                        for c in range(KC):
                            for n2 in range(2):
                                nc.tensor.matmul(
                                    psh[:, n2 * 512:(n2 + 1) * 512],
                                    lhsT=xhT_s[:, KC + c, :],
                                    rhs=WnhT_s[:, c, n2 * 512:(n2 + 1) * 512],
                                    start=(c == 0), stop=(c == KC - 1))
                        hn = sm.tile([BL, H], BF, tag="hn")
                        nc.vector.tensor_tensor(
                            hn[:], psh[0:BL, :],
                            bhn_s[0:BL, :], ALU.add)

                        nc.scalar.activation(grz[:, 0:H], grz[:, 0:H], ACT.Sigmoid)
                        nc.scalar.activation(grz[:, H:2 * H], grz[:, H:2 * H], ACT.Sigmoid)
                        n_t = sm.tile([BL, H], F32, tag="nt")
                        nc.vector.tensor_mul(n_t[:], grz[:, 0:H], hn[:])
                        nc.vector.tensor_add(n_t[:], n_t[:], inn[:])
                        nc.scalar.activation(n_t[:], n_t[:], ACT.Tanh)
                        hmn = sm.tile([BL, H], F32, tag="hmn")
                        nc.vector.tensor_sub(hmn[:], hm_s[:], n_t[:])
                        nc.vector.tensor_mul(hmn[:], grz[:, H:2 * H], hmn[:])
                        nc.vector.tensor_add(hm_s[:], n_t[:], hmn[:])
                        # hT for next step + HT_all
                        for c in range(KC):
                            nc.vector.tensor_copy(
                                stage_s[0:BL, :], hm_s[:, c * 128:(c + 1) * 128])
                            pt = ptr.tile([128, 128], F32, tag="tr")
                            nc.tensor.transpose(pt[:], stage_s[:], ident_s[:])
                            nc.vector.tensor_copy(xhT_s[:, KC + c, 0:BL], pt[:, 0:BL])
                            nc.vector.tensor_copy(
                                HTall_s[:, c, t * BL:(t + 1) * BL], pt[:, 0:BL])

            # ---- phase 3 ----
            nc.sync.dma_start(hout_d[:], hm_s[:])
            nc.sync.dma_start(hg_in[:], HTall_s[:])
            if K_CC:
                nc.gpsimd.collective_compute(
                    "AllGather", ALU.bypass,
                    replica_groups=[list(range(NC))],
                    ins=[hg_in.opt()], outs=[hg_out.opt()])
            else:
                for src_i in range(NC):
                    nc.sync.dma_start(
                        hg_out[src_i * 128:(src_i + 1) * 128], hg_in[:])

            with tc.tile_pool(name="att", bufs=2) as att:
                attn_sb = att.tile([BL, T, S], F32, tag="attn_out")
                azero = att.tile([128, 128], F32, tag="azero")
                nc.vector.memset(azero[:], 0.0)
                for t in range(T):
                    nc.vector.tensor_copy(azero[:, 0:BL], attnb_s[:, t, :])
                    pt = ptr.tile([128, 128], F32, tag="tr")
                    nc.tensor.transpose(pt[:], azero[:], ident_s[:])
                    nc.vector.tensor_copy(attn_sb[:, t, :], pt[0:BL, :])
                nc.sync.dma_start(attn_d[:], attn_sb[:])

            with tc.tile_pool(name="ph3", bufs=1) as ph3, \
                 tc.tile_pool(name="ph3s", bufs=3) as ph3s:
                outWT_s = ph3.tile([128, KC, VS], BF)
                load_T(outWT_s, outWT_d, KC)
                outb_s = ph3.tile([128, VS], F32)
                nc.sync.dma_start(outb_s[:], outb_d[0:1, :].partition_broadcast(128))
                HTg_s = ph3.tile([128, KC, NC, BL * T], BF)
                for src in range(NC):
                    for c in range(KC):
                        nc.sync.dma_start(
                            HTg_s[:, c, src, :],
                            hg_out[src * 128:(src + 1) * 128, c, :])
                logits_s = ph3.tile([128, MT, VS], BF)
                sumex_s = ph3.tile([128, MT], F32)
                nlse_s = ph3.tile([128, MT], F32)
                HTg_f = HTg_s[:].rearrange("p k c i -> p k (c i)")
                for m in range(MT):
                    for ncix in range(0, NCH, 2):
                        ps = psum.tile([128, H], F32, tag="mm")
                        for half in range(2):
                            nci = ncix + half
                            for c in range(KC):
                                nc.tensor.matmul(
                                    ps[:, half * 512:half * 512 + NW],
                                    lhsT=HTg_f[:, c, m * 128:(m + 1) * 128],
                                    rhs=outWT_s[:, c, nci * NW:(nci + 1) * NW],
                                    start=(c == 0), stop=(c == KC - 1))
                        for half in range(2):
                            nci = ncix + half
                            nc.vector.tensor_tensor(
                                logits_s[:, m, nci * NW:(nci + 1) * NW],
                                ps[:, half * 512:half * 512 + NW],
                                outb_s[:, nci * NW:(nci + 1) * NW],
                                ALU.add)
                            esc = ph3s.tile([128, NW], BF, tag="esc")
                            pex = ph3s.tile([128, 1], F32, tag="pex")
                            nc.scalar.activation(
                                esc[:], logits_s[:, m, nci * NW:(nci + 1) * NW],
                                ACT.Exp, accum_out=pex[:])
                            if nci == 0:
                                nc.vector.tensor_copy(sumex_s[:, m:m + 1], pex[:])
                            else:
                                nc.vector.tensor_add(
                                    sumex_s[:, m:m + 1], sumex_s[:, m:m + 1], pex[:])
                nc.sync.dma_start(st_in[:], sumex_s[:])
                if K_CC:
                    nc.gpsimd.collective_compute(
                        "AllReduce", ALU.add,
                        replica_groups=[list(range(NC))],
                        ins=[st_in.opt()], outs=[st_out.opt()])
                else:
                    nc.sync.dma_start(st_out[:], st_in[:])
                nc.sync.dma_start(nlse_s[:], st_out[:])
                nc.scalar.activation(nlse_s[:], nlse_s[:], ACT.Ln)
                nc.vector.tensor_scalar_mul(nlse_s[:], nlse_s[:], -1.0)
                for m in range(MT):
                    for nci in range(NCH):
                        fo = ph3s.tile([128, NW], F32, tag="fo")
                        nc.scalar.activation(
                            fo[:], logits_s[:, m, nci * NW:(nci + 1) * NW],
                            ACT.Identity, bias=nlse_s[:, m:m + 1])
                        nc.sync.dma_start(
                            logp_d[m * 128:(m + 1) * 128, nci * NW:(nci + 1) * NW],
                            fo[:])

    nc.compile()
    return nc


def _prep_inputs(inputs):
    enc = np.asarray(inputs["encoder_outputs"], np.float32)
    hidden = np.asarray(inputs["hidden"], np.float32)
    target = np.asarray(inputs["target"])
    emb_W = np.asarray(inputs["emb_W"], np.float32)
    Wa_w = np.asarray(inputs["Wa_w"], np.float32)
    Wa_b = np.asarray(inputs["Wa_b"], np.float32)
    Ua_w = np.asarray(inputs["Ua_w"], np.float32)
    Ua_b = np.asarray(inputs["Ua_b"], np.float32)
    Va_w = np.asarray(inputs["Va_w"], np.float32)
    W_ih = np.asarray(inputs["W_ih"], np.float32)
    W_hh = np.asarray(inputs["W_hh"], np.float32)
    b_ih = np.asarray(inputs["b_ih"], np.float32)
    b_hh = np.asarray(inputs["b_hh"], np.float32)
    out_W = np.asarray(inputs["out_W"], np.float32)
    out_b = np.asarray(inputs["out_b"], np.float32)

    tokens = np.concatenate(
        [np.zeros((B, 1), np.int64), target[:, : T - 1].astype(np.int64)], axis=1)

    bemb = (b_ih + b_hh).astype(np.float32)
    bemb[2 * H:] = b_ih[2 * H:]

    shared = {
        "UaT": _bf(Ua_w.T),
        "Uab": _bf((Ua_b + Wa_b)[None, :]),
        "WaT": _bf(Wa_w.T),
        "Va": _bf(Va_w[0][None, :]),
        "bemb": _bf(bemb[None, :]),
        "W1T": _bf(W_ih[:, :H].T),
        "WrzT": _bf(np.concatenate([W_ih[:2 * H, H:].T, W_hh[:2 * H, :].T], axis=0)),
        "WncT": _bf(W_ih[2 * H:, H:].T),
        "WnhT": _bf(W_hh[2 * H:, :].T),
        "bhn": b_hh[2 * H:][None, :].astype(np.float32),
        "ident": np.eye(128, dtype=np.float32),
        "identb": np.eye(128, dtype=np.float32).astype(NPBF),
    }
    bsel = np.zeros((128, BL, 128), np.float32)
    for b in range(BL):
        bsel[b, b, :] = 1.0
    shared["bsel"] = _bf(bsel)

    emb_bf = emb_W.astype(NPBF)
    in_maps = []
    for c in range(NC):
        bsl = slice(c * BL, (c + 1) * BL)
        enc_c = enc[bsl]
        toks = tokens[bsl].T.reshape(-1)          # [80], i = t*8 + b
        embT = np.ascontiguousarray(emb_bf[toks].astype(np.float32).T)  # [1024, 80]
        h0 = hidden[0, bsl]
        h0T = np.ascontiguousarray(h0.T)          # [1024, 8]
        m = dict(shared)
        m.update({
            "encT": _bf(enc_c.transpose(0, 2, 1)),
            "encN": _bf(enc_c),
            "outWT": _bf(out_W[c * VS:(c + 1) * VS, :].T),
            "outb": out_b[c * VS:(c + 1) * VS][None, :].astype(np.float32),
            "embT": _bf(embT.reshape(KC, 128, BL * T).transpose(1, 0, 2)),
            "h0T": _bf(h0T.reshape(KC, 128, BL).transpose(1, 0, 2)),
            "h0": h0.astype(np.float32),
        })
        in_maps.append(m)
    return in_maps


def _assemble(results):
    logp = np.concatenate(
        [results[c]["logp_s"].reshape(NC, T, BL, VS) for c in range(NC)], axis=3)
    logp = logp.transpose(0, 2, 1, 3).reshape(B, T, V)
    hT = np.concatenate([results[c]["h_out"] for c in range(NC)], axis=0)[None]
    attns = np.concatenate([results[c]["attn_out"] for c in range(NC)], axis=0)
    return logp, hT, attns


def kernel(**inputs):
    if "nc" not in _CACHE:
        _CACHE["nc"] = _build_program()
    nc = _CACHE["nc"]
    in_maps = _prep_inputs(inputs)
    res = run_bass_kernel_spmd(nc, in_maps, core_ids=list(range(NC)))
    return _assemble(res.results)


if __name__ == "__main__":
    import reference
    inputs = {k: np.asarray(v) for k, v in reference.setup_inputs().items()}
    outs = kernel(**inputs)
    print([o.shape for o in outs])


# revision 18
# speedup vs baseline: 1.0498x; 1.0498x over previous
"""Trainium2 Bass kernel for nn_AttnDecoderRNN (B=64,S=128,H=1024,V=32000,T=10).

Strategy (8 NeuronCores, SPMD, one NEFF):
- Recurrence (attention + GRU) batch-sharded: 8 batches/core, zero cross-core
  traffic during the 10 steps.
- Output projection vocab-sharded: 4000 vocab rows/core over all 640 (b,t)
  rows, with one AllGather (hidden states) + one AllReduce (softmax sums).
- bf16 matmuls with fp32 PSUM accumulation; log_softmax without max-shift
  (logits are O(1); exp is fp32-safe).
"""
import os
import sys

for p in ("/opt/trn_rl_repo", "/root/.axon_site/_ro/trn_rl_repo"):
    if os.path.isdir(p) and p not in sys.path:
        sys.path.insert(0, p)

import numpy as np
import ml_dtypes

import concourse.bass as bass
import concourse.mybir as mybir
import concourse.bacc as bacc
import concourse.tile as tile
import concourse.bass_isa as bass_isa
from concourse.bass_utils import run_bass_kernel_spmd

BF = mybir.dt.bfloat16
F32 = mybir.dt.float32
NPBF = ml_dtypes.bfloat16

B, S, H, V, T = 64, 128, 1024, 32000, 10
NC = 8
BL = B // NC          # 8 local batches per core
VS = V // NC          # 4000 vocab rows per core
KC = H // 128         # 8 k-chunks of hidden dim
MT = (B * T) // 128   # 5 row-tiles of the 640 global (b,t) rows
NCH = 8               # vocab-shard n-chunks
NW = VS // NCH        # 500 columns per chunk

ACT = mybir.ActivationFunctionType
ALU = mybir.AluOpType

_CACHE = {}
K_CC = os.environ.get("K_CC", "1") == "1"
K_CT = os.environ.get("K_CT", "1") == "1"



def _bf(x):
    return np.ascontiguousarray(x).astype(NPBF)


def _build_program():
    nc = bacc.Bacc("TRN2", target_bir_lowering=False, debug=False, num_devices=NC)

    def din(name, shape, dt=BF):
        return nc.dram_tensor(name, list(shape), dt, kind="ExternalInput").ap()

    def dout(name, shape, dt=F32):
        return nc.dram_tensor(name, list(shape), dt, kind="ExternalOutput").ap()

    encT_d = din("encT", [BL, H, S])            # Uk lhsT: [b][k][s]
    encN_d = din("encN", [BL, S, H])            # ctx rhs: [b][s][h]
    UaT_d = din("UaT", [H, H])
    Uab_d = din("Uab", [1, H])                  # Ua_b + Wa_b folded (bf16)
    WaT_d = din("WaT", [H, H])
    Va_d = din("Va", [1, H])
    W1T_d = din("W1T", [H, 3 * H])
    bemb_d = din("bemb", [1, 3 * H])            # b_ih+b_hh (rz), b_ih (n) (bf16)
    WrzT_d = din("WrzT", [2 * H, 2 * H])
    WncT_d = din("WncT", [H, H])
    WnhT_d = din("WnhT", [H, H])
    bhn_d = din("bhn", [1, H], F32)             # b_hh (n gate)
    outWT_d = din("outWT", [H, VS])
    outb_d = din("outb", [1, VS], F32)
    embT_d = din("embT", [128, KC, BL * T])     # pre-gathered embeddings
    h0T_d = din("h0T", [128, KC, BL])
    h0_d = din("h0", [BL, H], F32)
    ident_d = din("ident", [128, 128], F32)
    identb_d = din("identb", [128, 128])
    bsel_d = din("bsel", [128, BL, 128])

    logp_d = dout("logp_s", [B * T, VS])        # rows: src*80 + t*8 + b
    hout_d = dout("h_out", [BL, H])
    attn_d = dout("attn_out", [BL, T, S])

    with tile.TileContext(nc) as tc:
        with tc.tile_pool(name="dram", bufs=1, space="DRAM") as dram, \
             tc.tile_pool(name="dataX", bufs=1) as dataX, \
             tc.tile_pool(name="psum", bufs=2, space="PSUM") as psum, \
             tc.tile_pool(name="ptr", bufs=2, space="PSUM") as ptr:

            hg_in = dram.tile([128, KC, BL * T], BF)
            hg_out = nc.dram_tensor("hg_out", [NC * 128, KC, BL * T], BF, addr_space="Shared").ap()
            st_in = dram.tile([128, MT], F32)
            st_out = nc.dram_tensor("st_out", [128, MT], F32, addr_space="Shared").ap()
            uk_d = dram.tile([BL, 128, H], BF)

            HTall_s = dataX.tile([128, KC, BL * T], BF)
            attnb_s = dataX.tile([128, T, BL], F32)
            hm_s = dataX.tile([BL, H], F32)
            ident_s = dataX.tile([128, 128], F32)
            stage_s = dataX.tile([128, 128], F32)   # padded transpose input

            nc.sync.dma_start(ident_s[:], ident_d[:])
            nc.sync.dma_start(hm_s[:], h0_d[:])
            nc.vector.memset(stage_s[:], 0.0)

            def load_T(sb, dr, kc):  # [kc*128+p, n] dram -> [p, kc, n] sbuf
                for c in range(kc):
                    nc.sync.dma_start(sb[:, c, :], dr[c * 128:(c + 1) * 128, :])

            with tc.tile_pool(name="ph2w", bufs=1) as ph2w, \
                 tc.tile_pool(name="data2", bufs=1) as data2:
                WaT_s = ph2w.tile([128, KC, H], BF)
                WrzT_s = ph2w.tile([128, 2 * KC, 2 * H], BF)
                WncT_s = ph2w.tile([128, KC, H], BF)
                WnhT_s = ph2w.tile([128, KC, H], BF)
                Va_s = ph2w.tile([128, H], BF)
                bhn_s = ph2w.tile([128, H], F32)
                embT_s = ph2w.tile([128, KC, BL * T], BF)

                load_T(WaT_s, WaT_d, KC)
                load_T(WrzT_s, WrzT_d, 2 * KC)
                load_T(WncT_s, WncT_d, KC)
                load_T(WnhT_s, WnhT_d, KC)
                nc.sync.dma_start(Va_s[:], Va_d[0:1, :].partition_broadcast(128))
                nc.sync.dma_start(bhn_s[:], bhn_d[0:1, :].partition_broadcast(128))
                nc.sync.dma_start(embT_s[:], embT_d[:])

                encN_s = data2.tile([128, BL, H], BF)
                identb_s = data2.tile([128, 128], BF)
                qsb_s = data2.tile([128, H], BF)
                bsel_s = data2.tile([128, BL, 128], BF)
                gie_s = data2.tile([128, 3 * H], BF)
                xhT_s = data2.tile([128, 2 * KC, 128], BF)
                ctxp_s = data2.tile([128, H], F32)
                wsel_s = data2.tile([128, BL, 128], BF)

                for b in range(BL):
                    nc.sync.dma_start(encN_s[:, b, :], encN_d[b])
                nc.sync.dma_start(identb_s[:], identb_d[:])
                nc.sync.dma_start(bsel_s[:], bsel_d[:])
                nc.sync.dma_start(xhT_s[:, KC:2 * KC, 0:BL], h0T_d[:])
                nc.vector.memset(xhT_s[:, 0:KC, :], 0.0)
                nc.vector.memset(xhT_s[:, KC:2 * KC, BL:128], 0.0)
                nc.vector.memset(ctxp_s[:], 0.0)
                nc.vector.memset(qsb_s[:], 0.0)
                nc.vector.memset(wsel_s[:], 0.0)

                # ---- phase 1: Uk (to DRAM) + gi_emb ----
                with tc.tile_pool(name="ph1", bufs=2) as ph1, \
                     tc.tile_pool(name="ph1w", bufs=1) as ph1w, \
                     tc.tile_pool(name="ph1s", bufs=4) as ph1s:
                    UaT_s = ph1w.tile([128, KC, H], BF)
                    Uab_s = ph1w.tile([128, H], BF)
                    bemb_s = ph1w.tile([128, 3 * H], BF)
                    load_T(UaT_s, UaT_d, KC)
                    nc.sync.dma_start(Uab_s[:], Uab_d[0:1, :].partition_broadcast(128))
                    nc.sync.dma_start(bemb_s[:], bemb_d[0:1, :].partition_broadcast(128))
                    for b in range(BL):
                        eT = ph1.tile([128, KC, S], BF, tag="encT")
                        for c in range(KC):
                            nc.sync.dma_start(
                                eT[:, c, :], encT_d[b, c * 128:(c + 1) * 128, :])
                        ps = psum.tile([128, H], F32, tag="mm")
                        for c in range(KC):
                            for n2 in range(2):
                                nc.tensor.matmul(
                                    ps[:, n2 * 512:(n2 + 1) * 512],
                                    lhsT=eT[:, c, :],
                                    rhs=UaT_s[:, c, n2 * 512:(n2 + 1) * 512],
                                    start=(c == 0), stop=(c == KC - 1))
                        uksb = ph1.tile([128, H], BF, tag="uksb")
                        nc.vector.tensor_tensor(
                            uksb[:], ps[:], Uab_s[:], ALU.add)
                        nc.sync.dma_start(uk_d[b], uksb[:])
                    # gi_emb[80, 3072] = embT.T @ W1T + bemb
                    for n3 in range(3):
                        ps = psum.tile([128, H], F32, tag="mm")
                        for half in range(2):
                            col0 = n3 * 1024 + half * 512
                            for c in range(KC):
                                w1 = ph1s.tile([128, 512], BF, tag="w1")
                                nc.sync.dma_start(
                                    w1[:], W1T_d[c * 128:(c + 1) * 128, col0:col0 + 512])
                                nc.tensor.matmul(
                                    ps[0:BL * T, half * 512:(half + 1) * 512],
                                    lhsT=embT_s[:, c, :],
                                    rhs=w1[:],
                                    start=(c == 0), stop=(c == KC - 1))
                        nc.vector.tensor_tensor(
                            gie_s[0:BL * T, n3 * 1024:(n3 + 1) * 1024],
                            ps[0:BL * T, :],
                            bemb_s[0:BL * T, n3 * 1024:(n3 + 1) * 1024],
                            ALU.add)

                # ---- phase 2: 10 recurrence steps ----
                with tc.tile_pool(name="step", bufs=2) as step, \
                     tc.tile_pool(name="uks", bufs=3) as uks, \
                     tc.tile_pool(name="sm", bufs=1) as sm:
                    for t in range(T):
                        # q = h @ Wa^T
                        psq = psum.tile([128, H], F32, tag="mm")
                        if K_CT:
                            for c in range(KC):
                                j = c // 2
                                for n2 in range(2):
                                    nc.tensor.matmul(
                                        psq[32 * j:32 * j + BL, n2 * 512:(n2 + 1) * 512],
                                        lhsT=xhT_s[:, KC + c, 0:BL],
                                        rhs=WaT_s[:, c, n2 * 512:(n2 + 1) * 512],
                                        start=(c % 2 == 0), stop=(c % 2 == 1),
                                        tile_position=(0, 32 * j))
                            qt1 = sm.tile([BL, H], F32, tag="pa")
                            nc.vector.tensor_copy(qt1[:], psq[0:BL, :])
                            nc.vector.tensor_tensor(qt1[:], qt1[:], psq[32:32 + BL, :], ALU.add)
                            nc.vector.tensor_tensor(qt1[:], qt1[:], psq[64:64 + BL, :], ALU.add)
                            nc.vector.tensor_tensor(qsb_s[0:BL, :], qt1[:], psq[96:96 + BL, :], ALU.add)
                        else:
                            for c in range(KC):
                                for n2 in range(2):
                                    nc.tensor.matmul(
                                        psq[:, n2 * 512:(n2 + 1) * 512],
                                        lhsT=xhT_s[:, KC + c, :],
                                        rhs=WaT_s[:, c, n2 * 512:(n2 + 1) * 512],
                                        start=(c == 0), stop=(c == KC - 1))
                            nc.vector.tensor_copy(qsb_s[:], psq[:])
                        q_sb = qsb_s

                        scores = sm.tile([128, BL], F32, tag="sc")
                        for b in range(BL):
                            ukt = uks.tile([128, H], BF, tag="uk")
                            nc.sync.dma_start(ukt[:], uk_d[b])
                            pst = psum.tile([128, H], F32, tag="mm")
                            for n2 in range(2):
                                sl = slice(n2 * 512, (n2 + 1) * 512)
                                nc.tensor.matmul(
                                    pst[:, sl], lhsT=identb_s[:], rhs=ukt[:, sl],
                                    start=True, stop=False)
                                nc.tensor.matmul(
                                    pst[:, sl], lhsT=bsel_s[:, b, :], rhs=q_sb[:, sl],
                                    start=False, stop=True)
                            tt = step.tile([128, H], BF, tag="tt")
                            nc.scalar.activation(tt[:], pst[:], ACT.Tanh)
                            th = step.tile([128, H], BF, tag="th")
                            nc.vector.scalar_tensor_tensor(
                                out=th[:], in0=tt[:], scalar=1.0, in1=Va_s[:],
                                op0=ALU.mult, op1=ALU.mult,
                                accum_out=scores[:, b:b + 1])
                        # softmax over s (partitions) on gpsimd
                        smax = sm.tile([128, BL], F32, tag="smax")
                        nc.gpsimd.partition_all_reduce(
                            smax[:], scores[:], 128, bass_isa.ReduceOp.max)
                        esb = sm.tile([128, BL], F32, tag="esb")
                        nc.vector.tensor_sub(esb[:], scores[:], smax[:])
                        nc.scalar.activation(esb[:], esb[:], ACT.Exp)
                        ssum = sm.tile([128, BL], F32, tag="ssum")
                        nc.gpsimd.partition_all_reduce(
                            ssum[:], esb[:], 128, bass_isa.ReduceOp.add)
                        rcp = sm.tile([128, BL], F32, tag="rcp")
                        nc.vector.reciprocal(rcp[:], ssum[:])
                        nc.vector.tensor_mul(attnb_s[:, t, :], esb[:], rcp[:])
                        for b in range(BL):
                            nc.vector.tensor_copy(
                                wsel_s[:, b, b:b + 1], attnb_s[:, t, b:b + 1])

                        # ctx: 8 accumulating matmuls; selector column b puts
                        # w_b @ encN[b] in psum row b, zeros elsewhere.
                        psc = psum.tile([128, H], F32, tag="mm")
                        for b in range(BL):
                            for n2 in range(2):
                                nc.tensor.matmul(
                                    psc[:, n2 * 512:(n2 + 1) * 512],
                                    lhsT=wsel_s[:, b, :],
                                    rhs=encN_s[:, b, n2 * 512:(n2 + 1) * 512],
                                    start=(b == 0), stop=(b == BL - 1))
                        nc.vector.tensor_copy(ctxp_s[0:BL, :], psc[0:BL, :])
                        for c in range(KC):
                            pt = ptr.tile([128, 128], F32, tag="tr")
                            nc.tensor.transpose(
                                pt[:], ctxp_s[:, c * 128:(c + 1) * 128], ident_s[:])
                            nc.vector.tensor_copy(xhT_s[:, c, 0:BL], pt[:, 0:BL])

                        # GRU gates
                        gie_t = sm.tile([BL, 3 * H], BF, tag="gie_t")
                        nc.sync.dma_start(gie_t[:], gie_s[t * BL:(t + 1) * BL, :])
                        grz = sm.tile([BL, 2 * H], BF, tag="grz")
                        for h2 in range(2):
                            ps = psum.tile([128, H], F32, tag="mm")
                            if K_CT:
                                for c in range(2 * KC):
                                    j = c // 4
                                    for n2 in range(2):
                                        col = h2 * H + n2 * 512
                                        nc.tensor.matmul(
                                            ps[32 * j:32 * j + BL, n2 * 512:(n2 + 1) * 512],
                                            lhsT=xhT_s[:, c, 0:BL],
                                            rhs=WrzT_s[:, c, col:col + 512],
                                            start=(c % 4 == 0), stop=(c % 4 == 3),
                                            tile_position=(0, 32 * j))
                                gt1 = sm.tile([BL, H], F32, tag="pa")
                                nc.vector.tensor_copy(gt1[:], ps[0:BL, :])
                                nc.vector.tensor_tensor(gt1[:], gt1[:], ps[32:32 + BL, :], ALU.add)
                                nc.vector.tensor_tensor(gt1[:], gt1[:], ps[64:64 + BL, :], ALU.add)
                                nc.vector.tensor_tensor(gt1[:], gt1[:], ps[96:96 + BL, :], ALU.add)
                                nc.vector.tensor_tensor(
                                    grz[:, h2 * H:(h2 + 1) * H], gt1[:],
                                    gie_t[:, h2 * H:(h2 + 1) * H], ALU.add)
                            else:
                                for c in range(2 * KC):
                                    for n2 in range(2):
                                        col = h2 * H + n2 * 512
                                        nc.tensor.matmul(
                                            ps[:, n2 * 512:(n2 + 1) * 512],
                                            lhsT=xhT_s[:, c, :],
                                            rhs=WrzT_s[:, c, col:col + 512],
                                            start=(c == 0), stop=(c == 2 * KC - 1))
                                nc.vector.tensor_tensor(
                                    grz[:, h2 * H:(h2 + 1) * H], ps[0:BL, :],
                                    gie_t[:, h2 * H:(h2 + 1) * H],
                                    ALU.add)
                        psn = psum.tile([128, H], F32, tag="mm")
                        inn = sm.tile([BL, H], BF, tag="inn")
                        if K_CT:
                            for c in range(KC):
                                j = c // 2
                                for n2 in range(2):
                                    nc.tensor.matmul(
                                        psn[32 * j:32 * j + BL, n2 * 512:(n2 + 1) * 512],
                                        lhsT=xhT_s[:, c, 0:BL],
                                        rhs=WncT_s[:, c, n2 * 512:(n2 + 1) * 512],
                                        start=(c % 2 == 0), stop=(c % 2 == 1),
                                        tile_position=(0, 32 * j))
                            nt1 = sm.tile([BL, H], F32, tag="pa")
                            nc.vector.tensor_copy(nt1[:], psn[0:BL, :])
                            nc.vector.tensor_tensor(nt1[:], nt1[:], psn[32:32 + BL, :], ALU.add)
                            nc.vector.tensor_tensor(nt1[:], nt1[:], psn[64:64 + BL, :], ALU.add)
                            nc.vector.tensor_tensor(nt1[:], nt1[:], psn[96:96 + BL, :], ALU.add)
                            nc.vector.tensor_tensor(inn[:], nt1[:], gie_t[:, 2 * H:3 * H], ALU.add)
                        else:
                            for c in range(KC):
                                for n2 in range(2):
                                    nc.tensor.matmul(
                                        psn[:, n2 * 512:(n2 + 1) * 512],
                                        lhsT=xhT_s[:, c, :],
                                        rhs=WncT_s[:, c, n2 * 512:(n2 + 1) * 512],
                                        start=(c == 0), stop=(c == KC - 1))
                            nc.vector.tensor_tensor(
                                inn[:], psn[0:BL, :],
                                gie_t[:, 2 * H:3 * H], ALU.add)
                        psh = psum.tile([128, H], F32, tag="mm")
                        hn = sm.tile([BL, H], BF, tag="hn")
                        if K_CT:
                            for c in range(KC):
                                j = c // 2
                                for n2 in range(2):
                                    nc.tensor.matmul(
                                        psh[32 * j:32 * j + BL, n2 * 512:(n2 + 1) * 512],
                                        lhsT=xhT_s[:, KC + c, 0:BL],
                                        rhs=WnhT_s[:, c, n2 * 512:(n2 + 1) * 512],
                                        start=(c % 2 == 0), stop=(c % 2 == 1),
                                        tile_position=(0, 32 * j))
                            ht1 = sm.tile([BL, H], F32, tag="pa")
                            nc.vector.tensor_copy(ht1[:], psh[0:BL, :])
                            nc.vector.tensor_tensor(ht1[:], ht1[:], psh[32:32 + BL, :], ALU.add)
                            nc.vector.tensor_tensor(ht1[:], ht1[:], psh[64:64 + BL, :], ALU.add)
                            nc.vector.tensor_tensor(ht1[:], ht1[:], psh[96:96 + BL, :], ALU.add)
                            nc.vector.tensor_tensor(hn[:], ht1[:], bhn_s[0:BL, :], ALU.add)
                        else:
                            for c in range(KC):
                                for n2 in range(2):
                                    nc.tensor.matmul(
                                        psh[:, n2 * 512:(n2 + 1) * 512],
                                        lhsT=xhT_s[:, KC + c, :],
                                        rhs=WnhT_s[:, c, n2 * 512:(n2 + 1) * 512],
                                        start=(c == 0), stop=(c == KC - 1))
                            nc.vector.tensor_tensor(
                                hn[:], psh[0:BL, :],
                                bhn_s[0:BL, :], ALU.add)

                        nc.scalar.activation(grz[:, 0:H], grz[:, 0:H], ACT.Sigmoid)
                        nc.scalar.activation(grz[:, H:2 * H], grz[:, H:2 * H], ACT.Sigmoid)
                        n_t = sm.tile([BL, H], F32, tag="nt")
                        nc.vector.tensor_mul(n_t[:], grz[:, 0:H], hn[:])
                        nc.vector.tensor_add(n_t[:], n_t[:], inn[:])
                        nc.scalar.activation(n_t[:], n_t[:], ACT.Tanh)
                        hmn = sm.tile([BL, H], F32, tag="hmn")
                        nc.vector.tensor_sub(hmn[:], hm_s[:], n_t[:])
                        nc.vector.tensor_mul(hmn[:], grz[:, H:2 * H], hmn[:])
                        nc.vector.tensor_add(hm_s[:], n_t[:], hmn[:])
                        # hT for next step + HT_all
                        for c in range(KC):
                            nc.vector.tensor_copy(
                                stage_s[0:BL, :], hm_s[:, c * 128:(c + 1) * 128])
                            pt = ptr.tile([128, 128], F32, tag="tr")
                            nc.tensor.transpose(pt[:], stage_s[:], ident_s[:])
                            nc.vector.tensor_copy(xhT_s[:, KC + c, 0:BL], pt[:, 0:BL])
                            nc.vector.tensor_copy(
                                HTall_s[:, c, t * BL:(t + 1) * BL], pt[:, 0:BL])

            # ---- phase 3 ----
            nc.sync.dma_start(hout_d[:], hm_s[:])
            nc.sync.dma_start(hg_in[:], HTall_s[:])
            if K_CC:
                nc.gpsimd.collective_compute(
                    "AllGather", ALU.bypass,
                    replica_groups=[list(range(NC))],
                    ins=[hg_in.opt()], outs=[hg_out.opt()])
            else:
                for src_i in range(NC):
                    nc.sync.dma_start(
                        hg_out[src_i * 128:(src_i + 1) * 128], hg_in[:])

            with tc.tile_pool(name="att", bufs=2) as att:
                attn_sb = att.tile([BL, T, S], F32, tag="attn_out")
                azero = att.tile([128, 128], F32, tag="azero")
                nc.vector.memset(azero[:], 0.0)
                for t in range(T):
                    nc.vector.tensor_copy(azero[:, 0:BL], attnb_s[:, t, :])
                    pt = ptr.tile([128, 128], F32, tag="tr")
                    nc.tensor.transpose(pt[:], azero[:], ident_s[:])
                    nc.vector.tensor_copy(attn_sb[:, t, :], pt[0:BL, :])
                nc.sync.dma_start(attn_d[:], attn_sb[:])

            with tc.tile_pool(name="ph3", bufs=1) as ph3, \
                 tc.tile_pool(name="ph3s", bufs=3) as ph3s:
                outWT_s = ph3.tile([128, KC, VS], BF)
                load_T(outWT_s, outWT_d, KC)
                outb_s = ph3.tile([128, VS], F32)
                nc.sync.dma_start(outb_s[:], outb_d[0:1, :].partition_broadcast(128))
                HTg_s = ph3.tile([128, KC, NC, BL * T], BF)
                for src in range(NC):
                    for c in range(KC):
                        nc.sync.dma_start(
                            HTg_s[:, c, src, :],
                            hg_out[src * 128:(src + 1) * 128, c, :])
                logits_s = ph3.tile([128, MT, VS], BF)
                sumex_s = ph3.tile([128, MT], F32)
                nlse_s = ph3.tile([128, MT], F32)
                HTg_f = HTg_s[:].rearrange("p k c i -> p k (c i)")
                for m in range(MT):
                    for ncix in range(0, NCH, 2):
                        ps = psum.tile([128, H], F32, tag="mm")
                        for half in range(2):
                            nci = ncix + half
                            for c in range(KC):
                                nc.tensor.matmul(
                                    ps[:, half * 512:half * 512 + NW],
                                    lhsT=HTg_f[:, c, m * 128:(m + 1) * 128],
                                    rhs=outWT_s[:, c, nci * NW:(nci + 1) * NW],
                                    start=(c == 0), stop=(c == KC - 1))
                        for half in range(2):
                            nci = ncix + half
                            nc.vector.tensor_tensor(
                                logits_s[:, m, nci * NW:(nci + 1) * NW],
                                ps[:, half * 512:half * 512 + NW],
                                outb_s[:, nci * NW:(nci + 1) * NW],
                                ALU.add)
                            esc = ph3s.tile([128, NW], BF, tag="esc")
                            pex = ph3s.tile([128, 1], F32, tag="pex")
                            nc.scalar.activation(
                                esc[:], logits_s[:, m, nci * NW:(nci + 1) * NW],
                                ACT.Exp, accum_out=pex[:])
                            if nci == 0:
                                nc.vector.tensor_copy(sumex_s[:, m:m + 1], pex[:])
                            else:
                                nc.vector.tensor_add(
                                    sumex_s[:, m:m + 1], sumex_s[:, m:m + 1], pex[:])
                nc.sync.dma_start(st_in[:], sumex_s[:])
                if K_CC:
                    nc.gpsimd.collective_compute(
                        "AllReduce", ALU.add,
                        replica_groups=[list(range(NC))],
                        ins=[st_in.opt()], outs=[st_out.opt()])
                else:
                    nc.sync.dma_start(st_out[:], st_in[:])
                nc.sync.dma_start(nlse_s[:], st_out[:])
                nc.scalar.activation(nlse_s[:], nlse_s[:], ACT.Ln)
                nc.vector.tensor_scalar_mul(nlse_s[:], nlse_s[:], -1.0)
                for m in range(MT):
                    for nci in range(NCH):
                        fo = ph3s.tile([128, NW], F32, tag="fo")
                        nc.scalar.activation(
                            fo[:], logits_s[:, m, nci * NW:(nci + 1) * NW],
                            ACT.Identity, bias=nlse_s[:, m:m + 1])
                        nc.sync.dma_start(
                            logp_d[m * 128:(m + 1) * 128, nci * NW:(nci + 1) * NW],
                            fo[:])

    nc.compile()
    return nc


def _prep_inputs(inputs):
    enc = np.asarray(inputs["encoder_outputs"], np.float32)
    hidden = np.asarray(inputs["hidden"], np.float32)
    target = np.asarray(inputs["target"])
    emb_W = np.asarray(inputs["emb_W"], np.float32)
    Wa_w = np.asarray(inputs["Wa_w"], np.float32)
    Wa_b = np.asarray(inputs["Wa_b"], np.float32)
    Ua_w = np.asarray(inputs["Ua_w"], np.float32)
    Ua_b = np.asarray(inputs["Ua_b"], np.float32)
    Va_w = np.asarray(inputs["Va_w"], np.float32)
    W_ih = np.asarray(inputs["W_ih"], np.float32)
    W_hh = np.asarray(inputs["W_hh"], np.float32)
    b_ih = np.asarray(inputs["b_ih"], np.float32)
    b_hh = np.asarray(inputs["b_hh"], np.float32)
    out_W = np.asarray(inputs["out_W"], np.float32)
    out_b = np.asarray(inputs["out_b"], np.float32)

    tokens = np.concatenate(
        [np.zeros((B, 1), np.int64), target[:, : T - 1].astype(np.int64)], axis=1)

    bemb = (b_ih + b_hh).astype(np.float32)
    bemb[2 * H:] = b_ih[2 * H:]

    shared = {
        "UaT": _bf(Ua_w.T),
        "Uab": _bf((Ua_b + Wa_b)[None, :]),
        "WaT": _bf(Wa_w.T),
        "Va": _bf(Va_w[0][None, :]),
        "bemb": _bf(bemb[None, :]),
        "W1T": _bf(W_ih[:, :H].T),
        "WrzT": _bf(np.concatenate([W_ih[:2 * H, H:].T, W_hh[:2 * H, :].T], axis=0)),
        "WncT": _bf(W_ih[2 * H:, H:].T),
        "WnhT": _bf(W_hh[2 * H:, :].T),
        "bhn": b_hh[2 * H:][None, :].astype(np.float32),
        "ident": np.eye(128, dtype=np.float32),
        "identb": np.eye(128, dtype=np.float32).astype(NPBF),
    }
    bsel = np.zeros((128, BL, 128), np.float32)
    for b in range(BL):
        bsel[b, b, :] = 1.0
    shared["bsel"] = _bf(bsel)

    emb_bf = emb_W.astype(NPBF)
    in_maps = []
    for c in range(NC):
        bsl = slice(c * BL, (c + 1) * BL)
        enc_c = enc[bsl]
        toks = tokens[bsl].T.reshape(-1)          # [80], i = t*8 + b
        embT = np.ascontiguousarray(emb_bf[toks].astype(np.float32).T)  # [1024, 80]
        h0 = hidden[0, bsl]
        h0T = np.ascontiguousarray(h0.T)          # [1024, 8]
        m = dict(shared)
        m.update({
            "encT": _bf(enc_c.transpose(0, 2, 1)),
            "encN": _bf(enc_c),
            "outWT": _bf(out_W[c * VS:(c + 1) * VS, :].T),
            "outb": out_b[c * VS:(c + 1) * VS][None, :].astype(np.float32),
            "embT": _bf(embT.reshape(KC, 128, BL * T).transpose(1, 0, 2)),
            "h0T": _bf(h0T.reshape(KC, 128, BL).transpose(1, 0, 2)),
            "h0": h0.astype(np.float32),
        })
        in_maps.append(m)
    return in_maps


def _assemble(results):
    logp = np.concatenate(
        [results[c]["logp_s"].reshape(NC, T, BL, VS) for c in range(NC)], axis=3)
    logp = logp.transpose(0, 2, 1, 3).reshape(B, T, V)
    hT = np.concatenate([results[c]["h_out"] for c in range(NC)], axis=0)[None]
    attns = np.concatenate([results[c]["attn_out"] for c in range(NC)], axis=0)
    return logp, hT, attns


def kernel(**inputs):
    if "nc" not in _CACHE:
        _CACHE["nc"] = _build_program()
    nc = _CACHE["nc"]
    in_maps = _prep_inputs(inputs)
    res = run_bass_kernel_spmd(nc, in_maps, core_ids=list(range(NC)))
    return _assemble(res.results)


if __name__ == "__main__":
    import reference
    inputs = {k: np.asarray(v) for k, v in reference.setup_inputs().items()}
    outs = kernel(**inputs)
    print([o.shape for o in outs])
